# revision 13
# baseline (speedup 1.0000x reference)
"""Trainium2 Bass kernel for multi-lengthscale RBF kernel self-attention.

Reference computation (B=2, N=4096, D=128, 4 heads of 32):
  d2[b,i,j] = ||coords[b,i]-coords[b,j]||^2
  att_h = exp(-d2/ls_h^2) row-normalized (+1e-8), ls = [0.5,1,2,4]
  out = concat_h(att_h @ (features @ Wv[h] + bv[h])) @ Wo + bo

v2 design (8 cores, query rows sharded, 512 queries/core/batch):
  * Gram trick with SPLIT-BF16 (K=13) as v1: G[j,i] = 2 xj.xi - |xj|^2
    - |xi|^2 per j-block, one [128,512] bf16 matmul each.
  * QUAD processing: g is one [128, 2048] PSUM tile (4 banks, quad =
    4 j-blocks = 2 pairs); elementwise ACT/DVE ops span the full 2048
    cols, amortizing the ~0.3-0.45us/op fixed overheads.
  * fp8 (e4m3) DoubleRow attention for the two BROAD heads:
      e1 = exp(G/16) [ls=4] and e2 = exp(G/4) [ls=2] are written as fp8
      [128, 4, 512] (quad = 4 j-block planes); V for those heads is fp8
      (33 cols incl. ones, padded to 48 for the 16B plane-stride ISA
      rule). One DoubleRow matmul contracts a j-block PAIR (K=256),
      halving PE streaming for those heads. Sharp heads (ls=1, 0.5)
      stay bf16 (fp8 e/V would bust the error gate there).
  * Per-quad TYPE pattern balances ACT vs DVE vs PE:
      A: ACT e1_f8, e2_f8, e3_bf direct; DVE e4 chain (2 muls)
      B: ACT e1_f8, e2_bf; DVE e2_f8 cast + e3 = (e2^2)^2 + e4 chain
      C: like B but h2 attention consumes e2_bf via bf16 matmuls
         (no cast; +2 PE matmuls for that quad's pairs)
  * PSUM map (8 banks): g quad 4 + att tile [128, 2048] 4. The att tile
    rows 0-32 hold the 4 head accumulators ([33, 512] each, ones row 32
    = rowsums); rows 64-95 receive PE-broadcast reciprocal rowsums
    (ones[1,32] x rsum_recip[1,512] matmuls, replacing the slow gpsimd
    partition_broadcast); rows 0-127 cols 0-511 are reused late in the
    epilogue as the Wo-projection accumulator (emitted before the next
    batch's first attention matmuls to keep PE queue order safe).
  * Software pipelining as v1: att matmuls lag one quad behind the
    elementwise chain; batch epilogues overlap the next batch's stream.
  * Output stored [o, i] (transposed); host fixes layout + adds bo_eff.

Host does only O(N*D) marshalling: coord hi/lo augmentation, V = F@Wv
(+ones) in bf16 for heads ls=1,0.5 (and a bf16 copy for ls=2 used by
type-C quads) and fp8 for ls=4,2, bo_eff added at the end, final
transpose.
"""

import numpy as np

B = 2
N = 4096
NCORES = 8
NQ = N // NCORES          # 512 query rows per core per batch
P = 128                   # partitions / j-block size
NJB = N // P              # 32 j-blocks
NPAIR = NJB // 2          # 16 j-block pairs
NQUAD = NJB // 4          # 8 quads per batch
KG = 13                   # gram contraction rows (split-bf16)
VW = 33                   # V columns per head incl. ones column
VP = 48                   # fp8 V plane stride (16B aligned >= 33)
D = 128

# per-quad elementwise type pattern, one char per global quad (2B x 8)
TYPES = "ABCABABC" "ABCABABC"

_BUILT = {}


def _build():
    import concourse.bass as bass
    import concourse.bacc as bacc
    import concourse.mybir as mybir
    import concourse.tile as tile

    f32 = mybir.dt.float32
    bf16 = mybir.dt.bfloat16
    fp8 = mybir.dt.float8e4
    AF = mybir.ActivationFunctionType
    DR = mybir.MatmulPerfMode.DoubleRow

    nc = bacc.Bacc("TRN2", target_bir_lowering=False, debug=False,
                   enable_asserts=True, num_devices=NCORES)

    grama = nc.dram_tensor("grama", (B, KG, N), bf16, kind="ExternalInput").ap()
    gramr = nc.dram_tensor("gramr", (B, KG, NQ), bf16, kind="ExternalInput").ap()
    vall_d = nc.dram_tensor("vall", (B, P, NJB, 3, VW), bf16,
                            kind="ExternalInput").ap()
    va8_d = nc.dram_tensor("va8", (B, P, NPAIR, 2, 2, VP), fp8,
                           kind="ExternalInput").ap()
    wo4_d = nc.dram_tensor("wo4", (32, 4 * D), bf16, kind="ExternalInput").ap()
    one_d = nc.dram_tensor("one32", (1, 32), bf16, kind="ExternalInput").ap()
    outt = nc.dram_tensor("outt", (B, D, NQ), f32, kind="ExternalOutput").ap()

    with tile.TileContext(nc) as tc:
        with (
            tc.tile_pool(name="const", bufs=1) as cp,
            tc.tile_pool(name="elem", bufs=4) as ep,
            tc.tile_pool(name="mnp", bufs=1) as lp,
            tc.tile_pool(name="rsp", bufs=2) as rp,
            tc.tile_pool(name="outp", bufs=2) as op_,
            tc.tile_pool(name="gps", bufs=1, space="PSUM") as gp,
            tc.tile_pool(name="aps", bufs=1, space="PSUM") as ap_,
        ):
            ga = {}
            gr = {}
            va = {}
            v8 = {}
            GACH = 2
            VACH = 2
            for b in range(B):
                gr[b] = cp.tile([KG, NQ], bf16, tag=f"gr{b}", name=f"gr{b}")
                ga[b] = cp.tile([KG, N], bf16, tag=f"ga{b}", name=f"ga{b}")
                va[b] = cp.tile([P, NJB, 3, VW], bf16, tag=f"va{b}",
                                name=f"va{b}")
                v8[b] = cp.tile([P, NPAIR, 2, 2, VP], fp8, tag=f"v8{b}",
                                name=f"v8{b}")
            wo4_sb = cp.tile([32, 4 * D], bf16, tag="wo4")
            one32 = cp.tile([1, 32], bf16, tag="one32")
            # interleave big loads so early j-blocks arrive first
            wg = N // GACH
            wj = NJB // VACH
            wp = NPAIR // VACH
            nc.sync.dma_start(gr[0][:], gramr[0])
            nc.sync.dma_start(ga[0][:, 0:wg], grama[0][:, 0:wg])
            nc.sync.dma_start(wo4_sb[:], wo4_d)
            nc.sync.dma_start(one32[:], one_d)
            nc.sync.dma_start(gr[1][:], gramr[1])
            for c in range(VACH):
                nc.sync.dma_start(v8[0][:, c * wp:(c + 1) * wp],
                                  va8_d[0][:, c * wp:(c + 1) * wp])
                nc.sync.dma_start(va[0][:, c * wj:(c + 1) * wj],
                                  vall_d[0][:, c * wj:(c + 1) * wj])
                if 0 < c < GACH:
                    nc.sync.dma_start(ga[0][:, c * wg:(c + 1) * wg],
                                      grama[0][:, c * wg:(c + 1) * wg])
            for c in range(GACH):
                nc.sync.dma_start(ga[1][:, c * wg:(c + 1) * wg],
                                  grama[1][:, c * wg:(c + 1) * wg])
            for c in range(VACH):
                nc.sync.dma_start(v8[1][:, c * wp:(c + 1) * wp],
                                  va8_d[1][:, c * wp:(c + 1) * wp])
                nc.sync.dma_start(va[1][:, c * wj:(c + 1) * wj],
                                  vall_d[1][:, c * wj:(c + 1) * wj])

            def emit_elem(b, qi, kind, g, c0, npr):
                # elementwise chain for npr pairs of quad qi reading
                # g[:, c0:c0+npr*1024]; returns one map per pair
                W = npr * 2 * NQ
                gsl = g[:, c0:c0 + W]
                e1 = ep.tile([P, 2 * npr, NQ], fp8, tag="e1")
                nc.scalar.activation(e1[:], gsl, AF.Exp, scale=1.0 / 16.0)
                if kind == "A":
                    e2 = ep.tile([P, 2 * npr, NQ], fp8, tag="e2")
                    nc.scalar.activation(e2[:], gsl, AF.Exp, scale=0.25)
                    e3 = ep.tile([P, W], bf16, tag="e3")
                    nc.scalar.activation(e3[:], gsl, AF.Exp, scale=1.0)
                    h2op = e2
                else:
                    e2b = ep.tile([P, W], bf16, tag="e2b")
                    nc.scalar.activation(e2b[:], gsl, AF.Exp, scale=0.25)
                    if kind == "B":
                        e2 = ep.tile([P, 2 * npr, NQ], fp8, tag="e2")
                        nc.vector.tensor_copy(e2[:], e2b[:])
                        h2op = e2
                    else:
                        h2op = e2b
                    t3 = ep.tile([P, W], bf16, tag="t3")
                    nc.vector.tensor_mul(t3[:], e2b[:], e2b[:])
                    e3 = ep.tile([P, W], bf16, tag="e3")
                    nc.vector.tensor_mul(e3[:], t3[:], t3[:])
                t4 = ep.tile([P, W], bf16, tag="t4")
                nc.vector.tensor_mul(t4[:], e3[:], e3[:])
                e4 = ep.tile([P, W], bf16, tag="e4")
                nc.vector.tensor_mul(e4[:], t4[:], t4[:])
                return [{"e1": e1, "h2": h2op, "e3": e3, "e4": e4,
                         "kind": kind, "pp": pp} for pp in range(npr)]

            def emit_quad(b, qi, kind, split):
                g = gp.tile([P, 4 * NQ], f32, tag="g")
                maps = []
                if split:
                    for half in range(2):
                        for jbl in (2 * half, 2 * half + 1):
                            jb = 4 * qi + jbl
                            nc.tensor.matmul(g[:, NQ * jbl:NQ * (jbl + 1)],
                                             ga[b][:, P * jb:P * (jb + 1)],
                                             gr[b][:], start=True, stop=True)
                        maps += emit_elem(b, qi, kind, g,
                                          half * 2 * NQ, 1)
                else:
                    for jbl in range(4):
                        jb = 4 * qi + jbl
                        nc.tensor.matmul(g[:, NQ * jbl:NQ * (jbl + 1)],
                                         ga[b][:, P * jb:P * (jb + 1)],
                                         gr[b][:], start=True, stop=True)
                    maps = emit_elem(b, qi, kind, g, 0, 2)
                return maps

            def emit_att_pair(b, att, p, m):
                # one j-block pair's attention matmuls, heads in
                # e-readiness order (h3 DR, h2, h1, h0)
                pp = m["pp"]
                nc.tensor.matmul(
                    att[0:VW, NQ * 3:NQ * 4],
                    v8[b][:, p, 0, :, 0:VW],
                    m["e1"][:, 2 * pp:2 * pp + 2, :],
                    start=(p == 0), stop=(p == NPAIR - 1),
                    perf_mode=DR)
                if m["kind"] != "C":
                    nc.tensor.matmul(
                        att[0:VW, NQ * 2:NQ * 3],
                        v8[b][:, p, 1, :, 0:VW],
                        m["h2"][:, 2 * pp:2 * pp + 2, :],
                        start=(p == 0), stop=(p == NPAIR - 1),
                        perf_mode=DR, skip_group_check=True)
                else:
                    for lo in range(2):
                        jb = 2 * p + lo
                        nc.tensor.matmul(
                            att[0:VW, NQ * 2:NQ * 3],
                            va[b][:, jb, 2, :],
                            m["h2"][:, NQ * (2 * pp + lo):
                                    NQ * (2 * pp + lo + 1)],
                            start=(jb == 0), stop=(jb == NJB - 1),
                            skip_group_check=True)
                for lo in range(2):
                    jb = 2 * p + lo
                    nc.tensor.matmul(
                        att[0:VW, NQ * 1:NQ * 2],
                        va[b][:, jb, 0, :],
                        m["e3"][:, NQ * (2 * pp + lo):NQ * (2 * pp + lo + 1)],
                        start=(jb == 0), stop=(jb == NJB - 1))
                for lo in range(2):
                    jb = 2 * p + lo
                    nc.tensor.matmul(
                        att[0:VW, NQ * 0:NQ * 1],
                        va[b][:, jb, 1, :],
                        m["e4"][:, NQ * (2 * pp + lo):NQ * (2 * pp + lo + 1)],
                        start=(jb == 0), stop=(jb == NJB - 1))

            # PSUM scratch regions inside the att tile (rows 33-127 are
            # unused by the head accumulators): PE-broadcast targets per
            # head, all disjoint from the late Wo accumulator [0:128, 0:512]
            BC = {3: (64, NQ * 3), 2: (64, NQ * 2), 1: (64, NQ * 1),
                  0: (32, NQ * 1)}

            def emit_epilogue(b, att):
                # per head: rowsum -> reciprocal -> bf16 -> PE broadcast
                # -> SBUF copy -> normalize mul -> Wo projection matmul.
                # Interleaved so the PE gets projection work early.
                mn = lp.tile([32, 4 * NQ], bf16, tag="mn", name=f"mn{b}")
                rsbs = {}
                # h0 is normalized FIRST: the Wo projection accumulator
                # reuses PSUM [0:128, 0:512], which contains h0's att
                # region, and its start=True write zeroes it.
                for k, h in enumerate((0, 3, 1, 2)):
                    sl = slice(NQ * h, NQ * (h + 1))
                    s1 = rp.tile([1, NQ], f32, tag=f"s1_{h}",
                                 name=f"s1_{b}{h}")
                    if k % 2 == 0:
                        nc.scalar.activation(s1[:], att[32:33, sl], AF.Copy)
                    else:
                        nc.vector.tensor_copy(s1[:], att[32:33, sl])
                    rsr = rp.tile([1, NQ], f32, tag=f"rsr_{h}",
                                  name=f"rsr_{b}{h}")
                    nc.vector.reciprocal_approx_fast(rsr[:], s1[:])
                    rsbs[h] = rsr
                for h in (0, 3, 1, 2):
                    sl = slice(NQ * h, NQ * (h + 1))
                    rbs = rp.tile([32, NQ], f32, tag=f"rbs_{h}",
                                  name=f"rbs_{b}{h}")
                    nc.gpsimd.partition_broadcast(rbs[:], rsbs[h][:])
                    nc.vector.tensor_mul(mn[:, sl], att[0:32, sl], rbs[:])
                    if h != 0:
                        # projection starts only after mn_h0 has read its
                        # region; order h3 (start), h1, h2, then h0 last
                        nc.tensor.matmul(att[0:P, 0:NQ],
                                         wo4_sb[:, D * 3:D * 4]
                                         if h == 3 else
                                         wo4_sb[:, D * h:D * (h + 1)],
                                         mn[:, NQ * h:NQ * (h + 1)],
                                         start=(h == 3), stop=False,
                                         skip_group_check=True)
                nc.tensor.matmul(att[0:P, 0:NQ], wo4_sb[:, 0:D],
                                 mn[:, 0:NQ], start=False, stop=True,
                                 skip_group_check=True)
                osb = op_.tile([D, NQ], f32, tag="osb")
                nc.scalar.activation(osb[:], att[0:P, 0:NQ], AF.Copy)
                nc.sync.dma_start(outt[b], osb[:])

            # ---- main loop: elementwise is emitted per QUAD (cheap wide
            # ops) but attention matmuls drain per PAIR, LAGP pairs late,
            # so the PE always holds a fine-grained backlog that covers
            # the elementwise chain latency -- including across the batch
            # boundary, where the epilogue (norm + projection, which
            # reuses the att PSUM region) lands before the next batch's
            # first drained pair by construction (LAGP >= 4).
            # The first quad of each batch runs pair-granular elementwise
            # to halve the pipeline refill latency.
            LAGP = 4
            atts = {}
            pend = []
            todo_norm = []

            def drain_one():
                bb, p_, m = pend.pop(0)
                emit_att_pair(bb, atts[bb], p_, m)
                if p_ == NPAIR - 1:
                    todo_norm.append(bb)

            for b in range(B):
                atts[b] = ap_.tile([P, 4 * NQ], f32, tag="att",
                                   name=f"att{b}")
                for qi in range(NQUAD):
                    kind = TYPES[b * NQUAD + qi]
                    maps = emit_quad(b, qi, kind, split=(qi == 0))
                    for pp, m in enumerate(maps):
                        pend.append((b, 2 * qi + pp, m))
                        if len(pend) > LAGP:
                            drain_one()
                        if todo_norm:
                            bb = todo_norm.pop(0)
                            emit_epilogue(bb, atts[bb])
            while pend:
                drain_one()
            while todo_norm:
                bb = todo_norm.pop(0)
                emit_epilogue(bb, atts[bb])

    nc.compile()
    return nc


def _bf16(x):
    import ml_dtypes
    return np.asarray(x, np.float32).astype(ml_dtypes.bfloat16)


def _fp8(x):
    import ml_dtypes
    return np.asarray(x, np.float32).astype(ml_dtypes.float8_e4m3)


def _prep(features, coords, Wv, bv, Wo, bo):
    coords = np.asarray(coords, np.float32)
    features = np.asarray(features, np.float32)
    Wv = np.asarray(Wv, np.float32)
    bv = np.asarray(bv, np.float32)
    Wo = np.asarray(Wo, np.float32)
    bo = np.asarray(bo, np.float32)

    def split(x):
        hi = _bf16(x).astype(np.float32)
        lo = _bf16(x - hi).astype(np.float32)
        return hi, lo

    # G[j,i] = 2 xi.xj - |xj|^2 - |xi|^2 via 13 bf16 K-rows
    sq = (coords ** 2).sum(-1)
    one = np.ones_like(sq)
    rows_a, rows_r = [], []
    for c in range(3):
        ah, al = split(coords[..., c])
        bh, bl = split(2.0 * coords[..., c])
        rows_a += [ah, ah, al]
        rows_r += [bh, bl, bh]
    sh, sl_ = split(-sq)
    rows_a += [sh, sl_, one, one]
    rows_r += [one, one, sh, sl_]
    grama = _bf16(np.stack(rows_a, axis=1))            # [B,13,N]
    gramr = _bf16(np.stack(rows_r, axis=1))            # [B,13,N]

    # V per head (no bv: folded into bo_eff) with ones column
    v = np.einsum('bnd,hdk->bnhk', features, Wv)       # [B, N, 4, 32]
    vaug = np.concatenate([v, np.ones((B, N, 4, 1), np.float32)], axis=-1)
    # bf16 tensor: hslots = (h1, h0, h2-for-type-C)
    vb = vaug[:, :, (1, 0, 2), :]                       # [B, N, 3, 33]
    vb = vb.reshape(B, NJB, P, 3, VW).transpose(0, 2, 1, 3, 4)
    vall = _bf16(np.ascontiguousarray(vb))              # [B, P, NJB, 3, VW]

    # fp8 tensor: hslots = (h3, h2), pair planes, padded to VP=48
    v8 = vaug[:, :, (3, 2), :]                          # [B, N, 2, 33]
    v8 = v8.reshape(B, NPAIR, 2, P, 2, VW)              # [B,pair,plane,P,hs,k]
    v8p = np.zeros((B, NPAIR, 2, P, 2, VP), np.float32)
    v8p[..., 0:VW] = v8
    v8p = v8p.transpose(0, 3, 1, 4, 2, 5)               # [B,P,pair,hs,plane,k]
    va8 = _fp8(np.ascontiguousarray(v8p))               # [B,P,NPAIR,2,2,VP]

    # wo4[k, 128h+o] = Wo[32h+k, o] for the four K=32 projection matmuls
    wo4 = _bf16(np.ascontiguousarray(
        Wo.reshape(4, 32, D).transpose(1, 0, 2).reshape(32, 4 * D)))

    one32 = _bf16(np.ones((1, 32), np.float32))
    bo_eff = bo + bv.reshape(-1) @ Wo                  # [128]
    return grama, gramr, vall, va8, wo4, one32, bo_eff


def kernel(features, coords, Wv, bv, Wo, bo):
    from concourse import bass_utils

    grama, gramr, vall, va8, wo4, one32, bo_eff = _prep(
        features, coords, Wv, bv, Wo, bo)

    if "nc" not in _BUILT:
        _BUILT["nc"] = _build()
    nc = _BUILT["nc"]

    in_maps = []
    for c in range(NCORES):
        sl = slice(c * NQ, (c + 1) * NQ)
        in_maps.append({
            "grama": grama,
            "gramr": np.ascontiguousarray(gramr[:, :, sl]),
            "vall": vall,
            "va8": va8,
            "wo4": wo4,
            "one32": one32,
        })
    res = bass_utils.run_bass_kernel_spmd(nc, in_maps,
                                          core_ids=list(range(NCORES)),
                                          trace=_BUILT.get("trace", False),
                                          tmpdir=_BUILT.get("tmpdir"))
    _BUILT["last_results"] = res

    out = np.empty((B, N, D), np.float32)
    for c in range(NCORES):
        ot = res.results[c]["outt"]                    # [B, 128, 512]
        for b in range(B):
            out[b, c * NQ:(c + 1) * NQ, :] = ot[b].T
    out += bo_eff[None, None, :]
    return out


# revision 17
# speedup vs baseline: 1.0269x; 1.0269x over previous
"""Trainium2 Bass kernel for multi-lengthscale RBF kernel self-attention.

Reference computation (B=2, N=4096, D=128, 4 heads of 32):
  d2[b,i,j] = ||coords[b,i]-coords[b,j]||^2
  att_h = exp(-d2/ls_h^2) row-normalized (+1e-8), ls = [0.5,1,2,4]
  out = concat_h(att_h @ (features @ Wv[h] + bv[h])) @ Wo + bo

v2 design (8 cores, query rows sharded, 512 queries/core/batch):
  * Gram trick with SPLIT-BF16 (K=13) as v1: G[j,i] = 2 xj.xi - |xj|^2
    - |xi|^2 per j-block, one [128,512] bf16 matmul each.
  * QUAD processing: g is one [128, 2048] PSUM tile (4 banks, quad =
    4 j-blocks = 2 pairs); elementwise ACT/DVE ops span the full 2048
    cols, amortizing the ~0.3-0.45us/op fixed overheads.
  * fp8 (e4m3) DoubleRow attention for the two BROAD heads:
      e1 = exp(G/16) [ls=4] and e2 = exp(G/4) [ls=2] are written as fp8
      [128, 4, 512] (quad = 4 j-block planes); V for those heads is fp8
      (33 cols incl. ones, padded to 48 for the 16B plane-stride ISA
      rule). One DoubleRow matmul contracts a j-block PAIR (K=256),
      halving PE streaming for those heads. Sharp heads (ls=1, 0.5)
      stay bf16 (fp8 e/V would bust the error gate there).
  * Per-quad TYPE pattern balances ACT vs DVE vs PE:
      A: ACT e1_f8, e2_f8, e3_bf direct; DVE e4 chain (2 muls)
      B: ACT e1_f8, e2_bf; DVE e2_f8 cast + e3 = (e2^2)^2 + e4 chain
      C: like B but h2 attention consumes e2_bf via bf16 matmuls
         (no cast; +2 PE matmuls for that quad's pairs)
  * PSUM map (8 banks): g quad 4 + att tile [128, 2048] 4. The att tile
    rows 0-32 hold the 4 head accumulators ([33, 512] each, ones row 32
    = rowsums); rows 64-95 receive PE-broadcast reciprocal rowsums
    (ones[1,32] x rsum_recip[1,512] matmuls, replacing the slow gpsimd
    partition_broadcast); rows 0-127 cols 0-511 are reused late in the
    epilogue as the Wo-projection accumulator (emitted before the next
    batch's first attention matmuls to keep PE queue order safe).
  * Software pipelining as v1: att matmuls lag one quad behind the
    elementwise chain; batch epilogues overlap the next batch's stream.
  * Output stored [o, i] (transposed); host fixes layout + adds bo_eff.

Host does only O(N*D) marshalling: coord hi/lo augmentation, V = F@Wv
(+ones) in bf16 for heads ls=1,0.5 (and a bf16 copy for ls=2 used by
type-C quads) and fp8 for ls=4,2, bo_eff added at the end, final
transpose.
"""

import numpy as np

B = 2
N = 4096
NCORES = 8
NQ = N // NCORES          # 512 query rows per core per batch
P = 128                   # partitions / j-block size
NJB = N // P              # 32 j-blocks
NPAIR = NJB // 2          # 16 j-block pairs
NQUAD = NJB // 4          # 8 quads per batch
KG = 13                   # gram contraction rows (split-bf16)
VW = 33                   # V columns per head incl. ones column
VP = 48                   # fp8 V plane stride (16B aligned >= 33)
D = 128

# per-quad elementwise type pattern, one char per global quad (2B x 8)
TYPES = "ABCABABC" "ABCABABC"

_BUILT = {}


def _build():
    import concourse.bass as bass
    import concourse.bacc as bacc
    import concourse.mybir as mybir
    import concourse.tile as tile

    f32 = mybir.dt.float32
    bf16 = mybir.dt.bfloat16
    fp8 = mybir.dt.float8e4
    AF = mybir.ActivationFunctionType
    DR = mybir.MatmulPerfMode.DoubleRow

    nc = bacc.Bacc("TRN2", target_bir_lowering=False, debug=False,
                   enable_asserts=True, num_devices=NCORES)

    grama = nc.dram_tensor("grama", (B, KG, N), bf16, kind="ExternalInput").ap()
    gramr = nc.dram_tensor("gramr", (B, KG, NQ), bf16, kind="ExternalInput").ap()
    vall_d = nc.dram_tensor("vall", (B, P, NJB, 3, VW), bf16,
                            kind="ExternalInput").ap()
    va8_d = nc.dram_tensor("va8", (B, P, NPAIR, 2, 2, VP), fp8,
                           kind="ExternalInput").ap()
    wo4_d = nc.dram_tensor("wo4", (32, 4 * D), bf16, kind="ExternalInput").ap()
    one_d = nc.dram_tensor("one32", (1, 32), bf16, kind="ExternalInput").ap()
    outt = nc.dram_tensor("outt", (B, D, NQ), f32, kind="ExternalOutput").ap()

    with tile.TileContext(nc) as tc:
        with (
            tc.tile_pool(name="const", bufs=1) as cp,
            tc.tile_pool(name="elem", bufs=6) as ep,
            tc.tile_pool(name="mnp", bufs=1) as lp,
            tc.tile_pool(name="rsp", bufs=1) as rp,
            tc.tile_pool(name="outp", bufs=1) as op_,
            tc.tile_pool(name="gps", bufs=1, space="PSUM") as gp,
            tc.tile_pool(name="aps", bufs=1, space="PSUM") as ap_,
        ):
            ga = {}
            gr = {}
            va = {}
            v8 = {}
            GACH = 2
            VACH = 2
            for b in range(B):
                gr[b] = cp.tile([KG, NQ], bf16, tag=f"gr{b}", name=f"gr{b}")
                ga[b] = cp.tile([KG, N], bf16, tag=f"ga{b}", name=f"ga{b}")
                va[b] = cp.tile([P, NJB, 3, VW], bf16, tag=f"va{b}",
                                name=f"va{b}")
                v8[b] = cp.tile([P, NPAIR, 2, 2, VP], fp8, tag=f"v8{b}",
                                name=f"v8{b}")
            wo4_sb = cp.tile([32, 4 * D], bf16, tag="wo4")
            # interleave big loads so early j-blocks arrive first
            wg = N // GACH
            wj = NJB // VACH
            wp = NPAIR // VACH
            nc.sync.dma_start(gr[0][:], gramr[0])
            nc.sync.dma_start(ga[0][:, 0:wg], grama[0][:, 0:wg])
            nc.sync.dma_start(wo4_sb[:], wo4_d)
            nc.sync.dma_start(gr[1][:], gramr[1])
            for c in range(VACH):
                nc.sync.dma_start(v8[0][:, c * wp:(c + 1) * wp],
                                  va8_d[0][:, c * wp:(c + 1) * wp])
                nc.sync.dma_start(va[0][:, c * wj:(c + 1) * wj],
                                  vall_d[0][:, c * wj:(c + 1) * wj])
                if 0 < c < GACH:
                    nc.sync.dma_start(ga[0][:, c * wg:(c + 1) * wg],
                                      grama[0][:, c * wg:(c + 1) * wg])
            for c in range(GACH):
                nc.sync.dma_start(ga[1][:, c * wg:(c + 1) * wg],
                                  grama[1][:, c * wg:(c + 1) * wg])
            for c in range(VACH):
                nc.sync.dma_start(v8[1][:, c * wp:(c + 1) * wp],
                                  va8_d[1][:, c * wp:(c + 1) * wp])
                nc.sync.dma_start(va[1][:, c * wj:(c + 1) * wj],
                                  vall_d[1][:, c * wj:(c + 1) * wj])

            def emit_elem(b, qi, kind, g, c0, npr):
                # elementwise chain for npr pairs of quad qi reading
                # g[:, c0:c0+npr*1024]; returns one map per pair
                W = npr * 2 * NQ
                gsl = g[:, c0:c0 + W]
                e1 = ep.tile([P, 2 * npr, NQ], fp8, tag="e1")
                nc.scalar.activation(e1[:], gsl, AF.Exp, scale=1.0 / 16.0)
                if kind == "A":
                    e2 = ep.tile([P, 2 * npr, NQ], fp8, tag="e2")
                    nc.scalar.activation(e2[:], gsl, AF.Exp, scale=0.25)
                    e3 = ep.tile([P, W], bf16, tag="e3")
                    nc.scalar.activation(e3[:], gsl, AF.Exp, scale=1.0)
                    h2op = e2
                else:
                    e2b = ep.tile([P, W], bf16, tag="e2b")
                    nc.scalar.activation(e2b[:], gsl, AF.Exp, scale=0.25)
                    if kind == "B":
                        e2 = ep.tile([P, 2 * npr, NQ], fp8, tag="e2")
                        nc.vector.tensor_copy(e2[:], e2b[:])
                        h2op = e2
                    else:
                        h2op = e2b
                    t3 = ep.tile([P, W], bf16, tag="t3")
                    nc.vector.tensor_mul(t3[:], e2b[:], e2b[:])
                    e3 = ep.tile([P, W], bf16, tag="e3")
                    nc.vector.tensor_mul(e3[:], t3[:], t3[:])
                t4 = ep.tile([P, W], bf16, tag="t4")
                nc.vector.tensor_mul(t4[:], e3[:], e3[:])
                e4 = ep.tile([P, W], bf16, tag="e4")
                nc.vector.tensor_mul(e4[:], t4[:], t4[:])
                return [{"e1": e1, "h2": h2op, "e3": e3, "e4": e4,
                         "kind": kind, "pp": pp} for pp in range(npr)]

            def emit_quad(b, qi, kind, split):
                g = gp.tile([P, 4 * NQ], f32, tag="g")
                maps = []
                if split:
                    for half in range(2):
                        for jbl in (2 * half, 2 * half + 1):
                            jb = 4 * qi + jbl
                            nc.tensor.matmul(g[:, NQ * jbl:NQ * (jbl + 1)],
                                             ga[b][:, P * jb:P * (jb + 1)],
                                             gr[b][:], start=True, stop=True)
                        maps += emit_elem(b, qi, kind, g,
                                          half * 2 * NQ, 1)
                else:
                    for jbl in range(4):
                        jb = 4 * qi + jbl
                        nc.tensor.matmul(g[:, NQ * jbl:NQ * (jbl + 1)],
                                         ga[b][:, P * jb:P * (jb + 1)],
                                         gr[b][:], start=True, stop=True)
                    maps = emit_elem(b, qi, kind, g, 0, 2)
                return maps

            def emit_att_pair(b, att, p, m):
                # one j-block pair's attention matmuls, heads in
                # e-readiness order (h3 DR, h2, h1, h0)
                pp = m["pp"]
                nc.tensor.matmul(
                    att[0:VW, NQ * 3:NQ * 4],
                    v8[b][:, p, 0, :, 0:VW],
                    m["e1"][:, 2 * pp:2 * pp + 2, :],
                    start=(p == 0), stop=(p == NPAIR - 1),
                    perf_mode=DR)
                if m["kind"] != "C":
                    nc.tensor.matmul(
                        att[0:VW, NQ * 2:NQ * 3],
                        v8[b][:, p, 1, :, 0:VW],
                        m["h2"][:, 2 * pp:2 * pp + 2, :],
                        start=(p == 0), stop=(p == NPAIR - 1),
                        perf_mode=DR, skip_group_check=True)
                else:
                    for lo in range(2):
                        jb = 2 * p + lo
                        nc.tensor.matmul(
                            att[0:VW, NQ * 2:NQ * 3],
                            va[b][:, jb, 2, :],
                            m["h2"][:, NQ * (2 * pp + lo):
                                    NQ * (2 * pp + lo + 1)],
                            start=(jb == 0), stop=(jb == NJB - 1),
                            skip_group_check=True)
                for lo in range(2):
                    jb = 2 * p + lo
                    nc.tensor.matmul(
                        att[0:VW, NQ * 1:NQ * 2],
                        va[b][:, jb, 0, :],
                        m["e3"][:, NQ * (2 * pp + lo):NQ * (2 * pp + lo + 1)],
                        start=(jb == 0), stop=(jb == NJB - 1))
                for lo in range(2):
                    jb = 2 * p + lo
                    nc.tensor.matmul(
                        att[0:VW, NQ * 0:NQ * 1],
                        va[b][:, jb, 1, :],
                        m["e4"][:, NQ * (2 * pp + lo):NQ * (2 * pp + lo + 1)],
                        start=(jb == 0), stop=(jb == NJB - 1))

            # PSUM scratch regions inside the att tile (rows 33-127 are
            # unused by the head accumulators): PE-broadcast targets per
            # head, all disjoint from the late Wo accumulator [0:128, 0:512]
            BC = {3: (64, NQ * 3), 2: (64, NQ * 2), 1: (64, NQ * 1),
                  0: (32, NQ * 1)}

            def emit_epilogue(b, att):
                # per head: rowsum -> reciprocal -> bf16 -> PE broadcast
                # -> SBUF copy -> normalize mul -> Wo projection matmul.
                # Interleaved so the PE gets projection work early.
                mn = lp.tile([32, 4 * NQ], bf16, tag="mn", name=f"mn{b}")
                rsbs = {}
                # h0 is normalized FIRST: the Wo projection accumulator
                # reuses PSUM [0:128, 0:512], which contains h0's att
                # region, and its start=True write zeroes it.
                for k, h in enumerate((0, 3, 1, 2)):
                    sl = slice(NQ * h, NQ * (h + 1))
                    s1 = rp.tile([1, NQ], f32, tag="s1",
                                 name=f"s1_{b}{h}")
                    if k % 2 == 0:
                        nc.scalar.activation(s1[:], att[32:33, sl], AF.Copy)
                    else:
                        nc.vector.tensor_copy(s1[:], att[32:33, sl])
                    rsr = rp.tile([1, NQ], f32, tag=f"rsr_{h}",
                                  name=f"rsr_{b}{h}")
                    nc.vector.reciprocal_approx_fast(rsr[:], s1[:])
                    rsbs[h] = rsr
                for h in (0, 3, 1, 2):
                    sl = slice(NQ * h, NQ * (h + 1))
                    rbs = rp.tile([32, NQ], f32, tag=f"rbs_{h}",
                                  name=f"rbs_{b}{h}")
                    nc.gpsimd.partition_broadcast(rbs[:], rsbs[h][:])
                    nc.vector.tensor_mul(mn[:, sl], att[0:32, sl], rbs[:])
                    if h != 0:
                        # projection starts only after mn_h0 has read its
                        # region; order h3 (start), h1, h2, then h0 last
                        nc.tensor.matmul(att[0:P, 0:NQ],
                                         wo4_sb[:, D * 3:D * 4]
                                         if h == 3 else
                                         wo4_sb[:, D * h:D * (h + 1)],
                                         mn[:, NQ * h:NQ * (h + 1)],
                                         start=(h == 3), stop=False,
                                         skip_group_check=True)
                nc.tensor.matmul(att[0:P, 0:NQ], wo4_sb[:, 0:D],
                                 mn[:, 0:NQ], start=False, stop=True,
                                 skip_group_check=True)
                osb = op_.tile([D, NQ], f32, tag="osb")
                nc.scalar.activation(osb[:], att[0:P, 0:NQ], AF.Copy)
                nc.sync.dma_start(outt[b], osb[:])

            # ---- main loop: elementwise is emitted per QUAD (cheap wide
            # ops) but attention matmuls drain per PAIR, LAGP pairs late,
            # so the PE always holds a fine-grained backlog that covers
            # the elementwise chain latency -- including across the batch
            # boundary, where the epilogue (norm + projection, which
            # reuses the att PSUM region) lands before the next batch's
            # first drained pair by construction (LAGP >= 4).
            # The first quad of each batch runs pair-granular elementwise
            # to halve the pipeline refill latency.
            LAGP = 6
            atts = {}
            pend = []
            todo_norm = []

            def drain_one():
                bb, p_, m = pend.pop(0)
                emit_att_pair(bb, atts[bb], p_, m)
                if p_ == NPAIR - 1:
                    todo_norm.append(bb)

            for b in range(B):
                atts[b] = ap_.tile([P, 4 * NQ], f32, tag="att",
                                   name=f"att{b}")
                for qi in range(NQUAD):
                    kind = TYPES[b * NQUAD + qi]
                    maps = emit_quad(b, qi, kind, split=(qi == 0))
                    for pp, m in enumerate(maps):
                        pend.append((b, 2 * qi + pp, m))
                        if len(pend) > LAGP:
                            drain_one()
                        if todo_norm:
                            bb = todo_norm.pop(0)
                            emit_epilogue(bb, atts[bb])
            while pend:
                drain_one()
            while todo_norm:
                bb = todo_norm.pop(0)
                emit_epilogue(bb, atts[bb])

    nc.compile()
    return nc


def _bf16(x):
    import ml_dtypes
    return np.asarray(x, np.float32).astype(ml_dtypes.bfloat16)


def _fp8(x):
    import ml_dtypes
    return np.asarray(x, np.float32).astype(ml_dtypes.float8_e4m3)


def _prep(features, coords, Wv, bv, Wo, bo):
    coords = np.asarray(coords, np.float32)
    features = np.asarray(features, np.float32)
    Wv = np.asarray(Wv, np.float32)
    bv = np.asarray(bv, np.float32)
    Wo = np.asarray(Wo, np.float32)
    bo = np.asarray(bo, np.float32)

    def split(x):
        hi = _bf16(x).astype(np.float32)
        lo = _bf16(x - hi).astype(np.float32)
        return hi, lo

    # G[j,i] = 2 xi.xj - |xj|^2 - |xi|^2 via 13 bf16 K-rows
    sq = (coords ** 2).sum(-1)
    one = np.ones_like(sq)
    rows_a, rows_r = [], []
    for c in range(3):
        ah, al = split(coords[..., c])
        bh, bl = split(2.0 * coords[..., c])
        rows_a += [ah, ah, al]
        rows_r += [bh, bl, bh]
    sh, sl_ = split(-sq)
    rows_a += [sh, sl_, one, one]
    rows_r += [one, one, sh, sl_]
    grama = _bf16(np.stack(rows_a, axis=1))            # [B,13,N]
    gramr = _bf16(np.stack(rows_r, axis=1))            # [B,13,N]

    # V per head (no bv: folded into bo_eff) with ones column
    v = np.einsum('bnd,hdk->bnhk', features, Wv)       # [B, N, 4, 32]
    vaug = np.concatenate([v, np.ones((B, N, 4, 1), np.float32)], axis=-1)
    # bf16 tensor: hslots = (h1, h0, h2-for-type-C)
    vb = vaug[:, :, (1, 0, 2), :]                       # [B, N, 3, 33]
    vb = vb.reshape(B, NJB, P, 3, VW).transpose(0, 2, 1, 3, 4)
    vall = _bf16(np.ascontiguousarray(vb))              # [B, P, NJB, 3, VW]

    # fp8 tensor: hslots = (h3, h2), pair planes, padded to VP=48
    v8 = vaug[:, :, (3, 2), :]                          # [B, N, 2, 33]
    v8 = v8.reshape(B, NPAIR, 2, P, 2, VW)              # [B,pair,plane,P,hs,k]
    v8p = np.zeros((B, NPAIR, 2, P, 2, VP), np.float32)
    v8p[..., 0:VW] = v8
    v8p = v8p.transpose(0, 3, 1, 4, 2, 5)               # [B,P,pair,hs,plane,k]
    va8 = _fp8(np.ascontiguousarray(v8p))               # [B,P,NPAIR,2,2,VP]

    # wo4[k, 128h+o] = Wo[32h+k, o] for the four K=32 projection matmuls
    wo4 = _bf16(np.ascontiguousarray(
        Wo.reshape(4, 32, D).transpose(1, 0, 2).reshape(32, 4 * D)))

    one32 = _bf16(np.ones((1, 32), np.float32))
    bo_eff = bo + bv.reshape(-1) @ Wo                  # [128]
    return grama, gramr, vall, va8, wo4, one32, bo_eff


def kernel(features, coords, Wv, bv, Wo, bo):
    from concourse import bass_utils

    grama, gramr, vall, va8, wo4, one32, bo_eff = _prep(
        features, coords, Wv, bv, Wo, bo)

    if "nc" not in _BUILT:
        _BUILT["nc"] = _build()
    nc = _BUILT["nc"]

    in_maps = []
    for c in range(NCORES):
        sl = slice(c * NQ, (c + 1) * NQ)
        in_maps.append({
            "grama": grama,
            "gramr": np.ascontiguousarray(gramr[:, :, sl]),
            "vall": vall,
            "va8": va8,
            "wo4": wo4,
            "one32": one32,
        })
    res = bass_utils.run_bass_kernel_spmd(nc, in_maps,
                                          core_ids=list(range(NCORES)),
                                          trace=_BUILT.get("trace", False),
                                          tmpdir=_BUILT.get("tmpdir"))
    _BUILT["last_results"] = res

    out = np.empty((B, N, D), np.float32)
    for c in range(NCORES):
        ot = res.results[c]["outt"]                    # [B, 128, 512]
        for b in range(B):
            out[b, c * NQ:(c + 1) * NQ, :] = ot[b].T
    out += bo_eff[None, None, :]
    return out


# revision 18
# speedup vs baseline: 1.0928x; 1.0641x over previous
"""Trainium2 Bass kernel for multi-lengthscale RBF kernel self-attention.

Reference computation (B=2, N=4096, D=128, 4 heads of 32):
  d2[b,i,j] = ||coords[b,i]-coords[b,j]||^2
  att_h = exp(-d2/ls_h^2) row-normalized (+1e-8), ls = [0.5,1,2,4]
  out = concat_h(att_h @ (features @ Wv[h] + bv[h])) @ Wo + bo

v2 design (8 cores, query rows sharded, 512 queries/core/batch):
  * Gram trick with SPLIT-BF16 (K=13) as v1: G[j,i] = 2 xj.xi - |xj|^2
    - |xi|^2 per j-block, one [128,512] bf16 matmul each.
  * QUAD processing: g is one [128, 2048] PSUM tile (4 banks, quad =
    4 j-blocks = 2 pairs); elementwise ACT/DVE ops span the full 2048
    cols, amortizing the ~0.3-0.45us/op fixed overheads.
  * fp8 (e4m3) DoubleRow attention for the two BROAD heads:
      e1 = exp(G/16) [ls=4] and e2 = exp(G/4) [ls=2] are written as fp8
      [128, 4, 512] (quad = 4 j-block planes); V for those heads is fp8
      (33 cols incl. ones, padded to 48 for the 16B plane-stride ISA
      rule). One DoubleRow matmul contracts a j-block PAIR (K=256),
      halving PE streaming for those heads. Sharp heads (ls=1, 0.5)
      stay bf16 (fp8 e/V would bust the error gate there).
  * Per-quad TYPE pattern balances ACT vs DVE vs PE:
      A: ACT e1_f8, e2_f8, e3_bf direct; DVE e4 chain (2 muls)
      B: ACT e1_f8, e2_bf; DVE e2_f8 cast + e3 = (e2^2)^2 + e4 chain
      C: like B but h2 attention consumes e2_bf via bf16 matmuls
         (no cast; +2 PE matmuls for that quad's pairs)
  * PSUM map (8 banks): g quad 4 + att tile [128, 2048] 4. The att tile
    rows 0-32 hold the 4 head accumulators ([33, 512] each, ones row 32
    = rowsums); rows 64-95 receive PE-broadcast reciprocal rowsums
    (ones[1,32] x rsum_recip[1,512] matmuls, replacing the slow gpsimd
    partition_broadcast); rows 0-127 cols 0-511 are reused late in the
    epilogue as the Wo-projection accumulator (emitted before the next
    batch's first attention matmuls to keep PE queue order safe).
  * Software pipelining as v1: att matmuls lag one quad behind the
    elementwise chain; batch epilogues overlap the next batch's stream.
  * Output stored [o, i] (transposed); host fixes layout + adds bo_eff.

Host does only O(N*D) marshalling: coord hi/lo augmentation, V = F@Wv
(+ones) in bf16 for heads ls=1,0.5 (and a bf16 copy for ls=2 used by
type-C quads) and fp8 for ls=4,2, bo_eff added at the end, final
transpose.
"""

import numpy as np

B = 2
N = 4096
NCORES = 8
NQ = N // NCORES          # 512 query rows per core per batch
P = 128                   # partitions / j-block size
NJB = N // P              # 32 j-blocks
NPAIR = NJB // 2          # 16 j-block pairs
NQUAD = NJB // 4          # 8 quads per batch
KG = 13                   # gram contraction rows (split-bf16)
VW = 33                   # V columns per head incl. ones column
VP = 48                   # fp8 V plane stride (16B aligned >= 33)
D = 128

# per-quad elementwise type pattern, one char per global quad (2B x 8)
TYPES = "EEEBEEEE" "EEEBEEEE"

_BUILT = {}


def _build():
    import concourse.bass as bass
    import concourse.bacc as bacc
    import concourse.mybir as mybir
    import concourse.tile as tile

    f32 = mybir.dt.float32
    bf16 = mybir.dt.bfloat16
    fp8 = mybir.dt.float8e4
    AF = mybir.ActivationFunctionType
    DR = mybir.MatmulPerfMode.DoubleRow

    nc = bacc.Bacc("TRN2", target_bir_lowering=False, debug=False,
                   enable_asserts=True, num_devices=NCORES)

    grama = nc.dram_tensor("grama", (B, KG, N), bf16, kind="ExternalInput").ap()
    gramr = nc.dram_tensor("gramr", (B, KG, NQ), bf16, kind="ExternalInput").ap()
    vall_d = nc.dram_tensor("vall", (B, P, NJB, 3, VW), bf16,
                            kind="ExternalInput").ap()
    va8_d = nc.dram_tensor("va8", (B, P, NPAIR, 2, 2, VP), fp8,
                           kind="ExternalInput").ap()
    wo4_d = nc.dram_tensor("wo4", (32, 4 * D), bf16, kind="ExternalInput").ap()
    one_d = nc.dram_tensor("one32", (1, 32), bf16, kind="ExternalInput").ap()
    outt = nc.dram_tensor("outt", (B, D, NQ), f32, kind="ExternalOutput").ap()

    with tile.TileContext(nc) as tc:
        with (
            tc.tile_pool(name="const", bufs=1) as cp,
            tc.tile_pool(name="elem", bufs=6) as ep,
            tc.tile_pool(name="mnp", bufs=1) as lp,
            tc.tile_pool(name="rsp", bufs=1) as rp,
            tc.tile_pool(name="outp", bufs=1) as op_,
            tc.tile_pool(name="gps", bufs=1, space="PSUM") as gp,
            tc.tile_pool(name="aps", bufs=1, space="PSUM") as ap_,
        ):
            ga = {}
            gr = {}
            va = {}
            v8 = {}
            GACH = 2
            VACH = 2
            for b in range(B):
                gr[b] = cp.tile([KG, NQ], bf16, tag=f"gr{b}", name=f"gr{b}")
                ga[b] = cp.tile([KG, N], bf16, tag=f"ga{b}", name=f"ga{b}")
                va[b] = cp.tile([P, NJB, 3, VW], bf16, tag=f"va{b}",
                                name=f"va{b}")
                v8[b] = cp.tile([P, NPAIR, 2, 2, VP], fp8, tag=f"v8{b}",
                                name=f"v8{b}")
            wo4_sb = cp.tile([32, 4 * D], bf16, tag="wo4")
            # interleave big loads so early j-blocks arrive first
            wg = N // GACH
            wj = NJB // VACH
            wp = NPAIR // VACH
            nc.sync.dma_start(gr[0][:], gramr[0])
            nc.sync.dma_start(ga[0][:, 0:wg], grama[0][:, 0:wg])
            nc.sync.dma_start(wo4_sb[:], wo4_d)
            nc.sync.dma_start(gr[1][:], gramr[1])
            for c in range(VACH):
                nc.sync.dma_start(v8[0][:, c * wp:(c + 1) * wp],
                                  va8_d[0][:, c * wp:(c + 1) * wp])
                nc.sync.dma_start(va[0][:, c * wj:(c + 1) * wj],
                                  vall_d[0][:, c * wj:(c + 1) * wj])
                if 0 < c < GACH:
                    nc.sync.dma_start(ga[0][:, c * wg:(c + 1) * wg],
                                      grama[0][:, c * wg:(c + 1) * wg])
            for c in range(GACH):
                nc.sync.dma_start(ga[1][:, c * wg:(c + 1) * wg],
                                  grama[1][:, c * wg:(c + 1) * wg])
            for c in range(VACH):
                nc.sync.dma_start(v8[1][:, c * wp:(c + 1) * wp],
                                  va8_d[1][:, c * wp:(c + 1) * wp])
                nc.sync.dma_start(va[1][:, c * wj:(c + 1) * wj],
                                  vall_d[1][:, c * wj:(c + 1) * wj])

            def emit_elem(b, qi, kind, g, c0, npr):
                # elementwise chain for npr pairs of quad qi reading
                # g[:, c0:c0+npr*1024]; returns one map per pair
                W = npr * 2 * NQ
                gsl = g[:, c0:c0 + W]
                e1 = ep.tile([P, 2 * npr, NQ], fp8, tag="e1")
                nc.scalar.activation(e1[:], gsl, AF.Exp, scale=1.0 / 16.0)
                if kind == "A":
                    e2 = ep.tile([P, 2 * npr, NQ], fp8, tag="e2")
                    nc.scalar.activation(e2[:], gsl, AF.Exp, scale=0.25)
                    e3 = ep.tile([P, W], bf16, tag="e3")
                    nc.scalar.activation(e3[:], gsl, AF.Exp, scale=1.0)
                    h2op = e2
                else:
                    e2b = ep.tile([P, W], bf16, tag="e2b")
                    nc.scalar.activation(e2b[:], gsl, AF.Exp, scale=0.25)
                    if kind in ("B", "E"):
                        e2 = ep.tile([P, 2 * npr, NQ], fp8, tag="e2")
                        if kind == "B":
                            nc.vector.tensor_copy(e2[:], e2b[:])
                        else:
                            # ACT cast from SBUF: g is released after only
                            # two PSUM reads, and this op overlaps the next
                            # quad's gram matmuls
                            nc.scalar.activation(e2[:], e2b[:], AF.Copy)
                        h2op = e2
                    else:
                        h2op = e2b
                    t3 = ep.tile([P, W], bf16, tag="t3")
                    nc.vector.tensor_mul(t3[:], e2b[:], e2b[:])
                    e3 = ep.tile([P, W], bf16, tag="e3")
                    nc.vector.tensor_mul(e3[:], t3[:], t3[:])
                t4 = ep.tile([P, W], bf16, tag="t4")
                nc.vector.tensor_mul(t4[:], e3[:], e3[:])
                e4 = ep.tile([P, W], bf16, tag="e4")
                nc.vector.tensor_mul(e4[:], t4[:], t4[:])
                return [{"e1": e1, "h2": h2op, "e3": e3, "e4": e4,
                         "kind": kind, "pp": pp} for pp in range(npr)]

            def emit_quad(b, qi, kind, split):
                g = gp.tile([P, 4 * NQ], f32, tag="g")
                maps = []
                if split:
                    for half in range(2):
                        for jbl in (2 * half, 2 * half + 1):
                            jb = 4 * qi + jbl
                            nc.tensor.matmul(g[:, NQ * jbl:NQ * (jbl + 1)],
                                             ga[b][:, P * jb:P * (jb + 1)],
                                             gr[b][:], start=True, stop=True)
                        maps += emit_elem(b, qi, kind, g,
                                          half * 2 * NQ, 1)
                else:
                    for jbl in range(4):
                        jb = 4 * qi + jbl
                        nc.tensor.matmul(g[:, NQ * jbl:NQ * (jbl + 1)],
                                         ga[b][:, P * jb:P * (jb + 1)],
                                         gr[b][:], start=True, stop=True)
                    maps = emit_elem(b, qi, kind, g, 0, 2)
                return maps

            def emit_att_pair(b, att, p, m):
                # one j-block pair's attention matmuls, heads in
                # e-readiness order (h3 DR, h2, h1, h0)
                pp = m["pp"]
                nc.tensor.matmul(
                    att[0:VW, NQ * 3:NQ * 4],
                    v8[b][:, p, 0, :, 0:VW],
                    m["e1"][:, 2 * pp:2 * pp + 2, :],
                    start=(p == 0), stop=(p == NPAIR - 1),
                    perf_mode=DR)
                if m["kind"] != "C":
                    nc.tensor.matmul(
                        att[0:VW, NQ * 2:NQ * 3],
                        v8[b][:, p, 1, :, 0:VW],
                        m["h2"][:, 2 * pp:2 * pp + 2, :],
                        start=(p == 0), stop=(p == NPAIR - 1),
                        perf_mode=DR, skip_group_check=True)
                else:
                    for lo in range(2):
                        jb = 2 * p + lo
                        nc.tensor.matmul(
                            att[0:VW, NQ * 2:NQ * 3],
                            va[b][:, jb, 2, :],
                            m["h2"][:, NQ * (2 * pp + lo):
                                    NQ * (2 * pp + lo + 1)],
                            start=(jb == 0), stop=(jb == NJB - 1),
                            skip_group_check=True)
                for lo in range(2):
                    jb = 2 * p + lo
                    nc.tensor.matmul(
                        att[0:VW, NQ * 1:NQ * 2],
                        va[b][:, jb, 0, :],
                        m["e3"][:, NQ * (2 * pp + lo):NQ * (2 * pp + lo + 1)],
                        start=(jb == 0), stop=(jb == NJB - 1))
                for lo in range(2):
                    jb = 2 * p + lo
                    nc.tensor.matmul(
                        att[0:VW, NQ * 0:NQ * 1],
                        va[b][:, jb, 1, :],
                        m["e4"][:, NQ * (2 * pp + lo):NQ * (2 * pp + lo + 1)],
                        start=(jb == 0), stop=(jb == NJB - 1))

            # PSUM scratch regions inside the att tile (rows 33-127 are
            # unused by the head accumulators): PE-broadcast targets per
            # head, all disjoint from the late Wo accumulator [0:128, 0:512]
            BC = {3: (64, NQ * 3), 2: (64, NQ * 2), 1: (64, NQ * 1),
                  0: (32, NQ * 1)}

            def emit_epilogue(b, att):
                # per head: rowsum -> reciprocal -> bf16 -> PE broadcast
                # -> SBUF copy -> normalize mul -> Wo projection matmul.
                # Interleaved so the PE gets projection work early.
                mn = lp.tile([32, 4 * NQ], bf16, tag="mn", name=f"mn{b}")
                rsbs = {}
                # h0 is normalized FIRST: the Wo projection accumulator
                # reuses PSUM [0:128, 0:512], which contains h0's att
                # region, and its start=True write zeroes it.
                for k, h in enumerate((0, 3, 1, 2)):
                    sl = slice(NQ * h, NQ * (h + 1))
                    s1 = rp.tile([1, NQ], f32, tag="s1",
                                 name=f"s1_{b}{h}")
                    if k % 2 == 0:
                        nc.scalar.activation(s1[:], att[32:33, sl], AF.Copy)
                    else:
                        nc.vector.tensor_copy(s1[:], att[32:33, sl])
                    rsr = rp.tile([1, NQ], f32, tag=f"rsr_{h}",
                                  name=f"rsr_{b}{h}")
                    nc.vector.reciprocal_approx_fast(rsr[:], s1[:])
                    rsbs[h] = rsr
                for h in (0, 3, 1, 2):
                    sl = slice(NQ * h, NQ * (h + 1))
                    rbs = rp.tile([32, NQ], f32, tag=f"rbs_{h}",
                                  name=f"rbs_{b}{h}")
                    nc.gpsimd.partition_broadcast(rbs[:], rsbs[h][:])
                    nc.vector.tensor_mul(mn[:, sl], att[0:32, sl], rbs[:])
                    if h != 0:
                        # projection starts only after mn_h0 has read its
                        # region; order h3 (start), h1, h2, then h0 last
                        nc.tensor.matmul(att[0:P, 0:NQ],
                                         wo4_sb[:, D * 3:D * 4]
                                         if h == 3 else
                                         wo4_sb[:, D * h:D * (h + 1)],
                                         mn[:, NQ * h:NQ * (h + 1)],
                                         start=(h == 3), stop=False,
                                         skip_group_check=True)
                nc.tensor.matmul(att[0:P, 0:NQ], wo4_sb[:, 0:D],
                                 mn[:, 0:NQ], start=False, stop=True,
                                 skip_group_check=True)
                osb = op_.tile([D, NQ], f32, tag="osb")
                nc.scalar.activation(osb[:], att[0:P, 0:NQ], AF.Copy)
                nc.sync.dma_start(outt[b], osb[:])

            # ---- main loop: elementwise is emitted per QUAD (cheap wide
            # ops) but attention matmuls drain per PAIR, LAGP pairs late,
            # so the PE always holds a fine-grained backlog that covers
            # the elementwise chain latency -- including across the batch
            # boundary, where the epilogue (norm + projection, which
            # reuses the att PSUM region) lands before the next batch's
            # first drained pair by construction (LAGP >= 4).
            # The first quad of each batch runs pair-granular elementwise
            # to halve the pipeline refill latency.
            LAGP = 6
            atts = {}
            pend = []
            todo_norm = []

            def drain_one():
                bb, p_, m = pend.pop(0)
                emit_att_pair(bb, atts[bb], p_, m)
                if p_ == NPAIR - 1:
                    todo_norm.append(bb)

            for b in range(B):
                atts[b] = ap_.tile([P, 4 * NQ], f32, tag="att",
                                   name=f"att{b}")
                for qi in range(NQUAD):
                    kind = TYPES[b * NQUAD + qi]
                    maps = emit_quad(b, qi, kind, split=(qi == 0))
                    for pp, m in enumerate(maps):
                        pend.append((b, 2 * qi + pp, m))
                        if len(pend) > LAGP:
                            drain_one()
                        if todo_norm:
                            bb = todo_norm.pop(0)
                            emit_epilogue(bb, atts[bb])
            while pend:
                drain_one()
            while todo_norm:
                bb = todo_norm.pop(0)
                emit_epilogue(bb, atts[bb])

    nc.compile()
    return nc


def _bf16(x):
    import ml_dtypes
    return np.asarray(x, np.float32).astype(ml_dtypes.bfloat16)


def _fp8(x):
    import ml_dtypes
    return np.asarray(x, np.float32).astype(ml_dtypes.float8_e4m3)


def _prep(features, coords, Wv, bv, Wo, bo):
    coords = np.asarray(coords, np.float32)
    features = np.asarray(features, np.float32)
    Wv = np.asarray(Wv, np.float32)
    bv = np.asarray(bv, np.float32)
    Wo = np.asarray(Wo, np.float32)
    bo = np.asarray(bo, np.float32)

    def split(x):
        hi = _bf16(x).astype(np.float32)
        lo = _bf16(x - hi).astype(np.float32)
        return hi, lo

    # G[j,i] = 2 xi.xj - |xj|^2 - |xi|^2 via 13 bf16 K-rows
    sq = (coords ** 2).sum(-1)
    one = np.ones_like(sq)
    rows_a, rows_r = [], []
    for c in range(3):
        ah, al = split(coords[..., c])
        bh, bl = split(2.0 * coords[..., c])
        rows_a += [ah, ah, al]
        rows_r += [bh, bl, bh]
    sh, sl_ = split(-sq)
    rows_a += [sh, sl_, one, one]
    rows_r += [one, one, sh, sl_]
    grama = _bf16(np.stack(rows_a, axis=1))            # [B,13,N]
    gramr = _bf16(np.stack(rows_r, axis=1))            # [B,13,N]

    # V per head (no bv: folded into bo_eff) with ones column
    v = np.einsum('bnd,hdk->bnhk', features, Wv)       # [B, N, 4, 32]
    vaug = np.concatenate([v, np.ones((B, N, 4, 1), np.float32)], axis=-1)
    # bf16 tensor: hslots = (h1, h0, h2-for-type-C)
    vb = vaug[:, :, (1, 0, 2), :]                       # [B, N, 3, 33]
    vb = vb.reshape(B, NJB, P, 3, VW).transpose(0, 2, 1, 3, 4)
    vall = _bf16(np.ascontiguousarray(vb))              # [B, P, NJB, 3, VW]

    # fp8 tensor: hslots = (h3, h2), pair planes, padded to VP=48
    v8 = vaug[:, :, (3, 2), :]                          # [B, N, 2, 33]
    v8 = v8.reshape(B, NPAIR, 2, P, 2, VW)              # [B,pair,plane,P,hs,k]
    v8p = np.zeros((B, NPAIR, 2, P, 2, VP), np.float32)
    v8p[..., 0:VW] = v8
    v8p = v8p.transpose(0, 3, 1, 4, 2, 5)               # [B,P,pair,hs,plane,k]
    va8 = _fp8(np.ascontiguousarray(v8p))               # [B,P,NPAIR,2,2,VP]

    # wo4[k, 128h+o] = Wo[32h+k, o] for the four K=32 projection matmuls
    wo4 = _bf16(np.ascontiguousarray(
        Wo.reshape(4, 32, D).transpose(1, 0, 2).reshape(32, 4 * D)))

    one32 = _bf16(np.ones((1, 32), np.float32))
    bo_eff = bo + bv.reshape(-1) @ Wo                  # [128]
    return grama, gramr, vall, va8, wo4, one32, bo_eff


def kernel(features, coords, Wv, bv, Wo, bo):
    from concourse import bass_utils

    grama, gramr, vall, va8, wo4, one32, bo_eff = _prep(
        features, coords, Wv, bv, Wo, bo)

    if "nc" not in _BUILT:
        _BUILT["nc"] = _build()
    nc = _BUILT["nc"]

    in_maps = []
    for c in range(NCORES):
        sl = slice(c * NQ, (c + 1) * NQ)
        in_maps.append({
            "grama": grama,
            "gramr": np.ascontiguousarray(gramr[:, :, sl]),
            "vall": vall,
            "va8": va8,
            "wo4": wo4,
            "one32": one32,
        })
    res = bass_utils.run_bass_kernel_spmd(nc, in_maps,
                                          core_ids=list(range(NCORES)),
                                          trace=_BUILT.get("trace", False),
                                          tmpdir=_BUILT.get("tmpdir"))
    _BUILT["last_results"] = res

    out = np.empty((B, N, D), np.float32)
    for c in range(NCORES):
        ot = res.results[c]["outt"]                    # [B, 128, 512]
        for b in range(B):
            out[b, c * NQ:(c + 1) * NQ, :] = ot[b].T
    out += bo_eff[None, None, :]
    return out
